# revision 12
# baseline (speedup 1.0000x reference)
"""Multi-head attention Trainium2 Bass kernel — iteration 2: paired heads.

Shapes (hardcoded): B=4, T=2048, E=1024, H=16, DK=64.
Sharding over 8 cores: core c -> (batch b = c//2, head-group g = c%2).

Key change vs iteration 1: heads are stored in PAIRS — qt2[p]/kt2[p]
[128, T] hold head 2p in partitions 0:64 and head 2p+1 in 64:128. The
two heads' score matmuls (contraction = 64) run CONCURRENTLY in the PE
array via row tiling (tile_position rows 0/64), halving score cost.
attn@V keeps the per-head M=65 ones-row form (denominator in PSUM row
64). Exp splits between ACT (exact) and DVE (Schraudolph int16/bf16
bit trick), tunable fraction.
"""

import numpy as np

import concourse.bass as bass
import concourse.tile as tile
from concourse import bacc, mybir
from concourse.bass_utils import run_bass_kernel_spmd

F32 = mybir.dt.float32
I16 = mybir.dt.int16
BF16 = mybir.dt.bfloat16
DT = BF16

B, T, E, H = 4, 2048, 1024, 16
DK = E // H            # 64
N_CORES = 8
FL = 512               # local f (8 heads * 64)
HL = 8                 # heads per core
NP = 4                 # head pairs per core
NT = T // 128          # 16 t-tiles
NE = E // 128          # 8 e-tiles
NFT = FL // 128        # 4 local f-tiles (one per head pair)
NC4 = T // 512         # 4 t-chunks of 512

SCHRA_A = 128.0 / np.log(2.0)
SCHRA_B = 16256.0 - 0.0579 * 128.0

# which key-tiles use the DVE Schraudolph exp (rest use exact ACT exp)
DVE_EXP_KS = frozenset((2, 5, 8, 11, 14))


def build_nc():
    nc = bacc.Bacc("TRN2", target_bir_lowering=False, debug=False,
                   enable_asserts=False)

    qT = nc.dram_tensor("qT", [E, T], DT, kind="ExternalInput").ap()
    kT = nc.dram_tensor("kT", [E, T], DT, kind="ExternalInput").ap()
    vT = nc.dram_tensor("vT", [E, T], DT, kind="ExternalInput").ap()
    wqT = nc.dram_tensor("wqT", [E, FL], DT, kind="ExternalInput").ap()
    wkT = nc.dram_tensor("wkT", [E, FL], DT, kind="ExternalInput").ap()
    wvT = nc.dram_tensor("wvT", [E, FL], DT, kind="ExternalInput").ap()
    woT = nc.dram_tensor("woT", [FL, E], DT, kind="ExternalInput").ap()
    bq = nc.dram_tensor("bq", [128, NFT], F32, kind="ExternalInput").ap()
    bk = nc.dram_tensor("bk", [128, NFT], F32, kind="ExternalInput").ap()
    bv2d = nc.dram_tensor("bv2d", [128, FL], DT, kind="ExternalInput").ap()
    bo2d = nc.dram_tensor("bo2d", [128, E], DT, kind="ExternalInput").ap()
    maskb = nc.dram_tensor("maskb", [128, NT], F32, kind="ExternalInput").ap()
    maskb16 = nc.dram_tensor("maskb16", [128, NT], F32,
                             kind="ExternalInput").ap()
    vones = nc.dram_tensor("vones", [128, HL], DT, kind="ExternalInput").ap()
    out = nc.dram_tensor("out", [T, E], DT, kind="ExternalOutput").ap()

    with tile.TileContext(nc) as tc:
        with (
            tc.tile_pool(name="const", bufs=1) as constp,
            tc.tile_pool(name="qkt", bufs=1) as qktp,
            tc.tile_pool(name="vsb", bufs=1) as vsbp,
            tc.tile_pool(name="xtl", bufs=1) as xtlp,
            tc.tile_pool(name="ps_s", bufs=6, space="PSUM") as ps_s,
            tc.tile_pool(name="ps_o", bufs=2, space="PSUM") as ps_o,
        ):
            # ---- constants (scalar ring keeps sync ring free) ----
            bq_sb = constp.tile([128, NFT], F32, tag="bq")
            nc.scalar.dma_start(out=bq_sb[:], in_=bq)
            bk_sb = constp.tile([128, NFT], F32, tag="bk")
            nc.scalar.dma_start(out=bk_sb[:], in_=bk)
            bv_sb = constp.tile([128, FL], DT, tag="bv2d")
            nc.scalar.dma_start(out=bv_sb[:], in_=bv2d)
            bo_sb = constp.tile([128, E], DT, tag="bo2d")
            nc.scalar.dma_start(out=bo_sb[:], in_=bo2d)
            mask_sb = constp.tile([128, NT], F32, tag="maskb")
            nc.scalar.dma_start(out=mask_sb[:], in_=maskb)
            mask16_sb = constp.tile([128, NT], F32, tag="maskb16")
            nc.scalar.dma_start(out=mask16_sb[:], in_=maskb16)

            # paired persistent activations: qt2[p]/kt2[p] rows 0:64 =
            # head 2p, rows 64:128 = head 2p+1 (f-tile p of the local 512)
            qt2 = [qktp.tile([128, T], DT, tag=f"qt{i}", name=f"qt{i}")
                   for i in range(NP)]
            kt2 = [qktp.tile([128, T], DT, tag=f"kt{i}", name=f"kt{i}")
                   for i in range(NP)]
            # V per t-tile: [128, 8 heads * 128]; per head: col 0 = 1.0
            # (row-sum trick: the softmax denominator lands in PSUM
            # partition 0, where reciprocal_approx_fast and
            # partition_broadcast work), cols 64..128 = V, rest zeros
            # (engine partition windows must be 0- or 64-based)
            vt = [vsbp.tile([128, HL * 128], DT, tag=f"v{j}", name=f"v{j}")
                  for j in range(NT)]
            for j in range(NT):
                nc.scalar.memzero(vt[j][:])
                nc.scalar.dma_start(
                    out=vt[j].rearrange("p (h w) -> p h w", w=128)[:, :, 0:1],
                    in_=vones.rearrange("p (h o) -> p h o", o=1))
            xtl = [xtlp.tile([128, T], DT, tag=f"x{i}", name=f"x{i}")
                   for i in range(NFT)]

            # ---- phase 1: K/Q projections, chunk-major (x loaded once) ----
            wp = tc.alloc_tile_pool(name="wqk", bufs=1)
            xlp = tc.alloc_tile_pool(name="xload", bufs=3)

            w_sb = {}
            for name, wdram in (("k", wkT), ("q", wqT)):
                w_sb[name] = [
                    wp.tile([128, FL], DT, tag=f"w{name}{e}",
                            name=f"w{name}{e}") for e in range(NE)]
                for e in range(NE):
                    nc.sync.dma_start(
                        out=w_sb[name][e][:],
                        in_=wdram[e * 128:(e + 1) * 128, :])

            for name in ("k", "q"):
                xdram = kT if name == "k" else qT
                bias_sb = bk_sb if name == "k" else bq_sb
                dst = kt2 if name == "k" else qt2
                for c in range(NC4):
                    xe = xlp.tile([128, NE * 512], DT, tag="xchunk",
                                  name="xchunk")
                    nc.sync.dma_start(
                        out=xe.rearrange("p (e t) -> p e t", e=NE),
                        in_=xdram.rearrange("(e p) t -> p e t", p=128)
                        [:, :, c * 512:(c + 1) * 512])
                    for f in range(4):
                        ps = ps_s.tile([128, 512], F32, tag="pss",
                                       name="psqk")
                        for e in range(NE):
                            nc.tensor.matmul(
                                ps[:],
                                lhsT=w_sb[name][e][:,
                                                   f * 128:(f + 1) * 128],
                                rhs=xe[:, e * 512:(e + 1) * 512],
                                start=(e == 0), stop=(e == NE - 1))
                        nc.scalar.add(
                            dst[f][:, c * 512:(c + 1) * 512],
                            ps[:], bias_sb[:, f:f + 1])

            # ---- phase 2: V projection (bias via DVE tensor_tensor) ----
            with tc.tile_pool(name="wv", bufs=1) as wvp, \
                 tc.tile_pool(name="vload", bufs=1) as vlp:
                wv_sb = [wvp.tile([128, FL], DT, tag=f"wv{e}", name=f"wv{e}")
                         for e in range(NE)]
                for e in range(NE):
                    nc.scalar.dma_start(out=wv_sb[e][:],
                                        in_=wvT[e * 128:(e + 1) * 128, :])
                bvr = bv_sb.rearrange("p (h w) -> p h w", w=64)
                for hf in range(2):
                    vf = [vlp.tile([128, 1024], DT, tag=f"vf{e}",
                                   name=f"vf{e}") for e in range(NE)]
                    for e in range(NE):
                        nc.sync.dma_start(
                            out=vf[e][:],
                            in_=vT[e * 128:(e + 1) * 128,
                                   hf * 1024:(hf + 1) * 1024])
                    for jj in range(NT // 2):
                        j = hf * (NT // 2) + jj
                        ps = ps_s.tile([128, FL], F32, tag="pss",
                                       name="psv")
                        for e in range(NE):
                            nc.tensor.matmul(
                                ps[:],
                                lhsT=vf[e][:, jj * 128:(jj + 1) * 128],
                                rhs=wv_sb[e][:],
                                start=(e == 0), stop=(e == NE - 1))
                        nc.vector.tensor_tensor(
                            out=vt[j].rearrange(
                                "p (h w) -> p h w", w=128)[:, :, 64:128],
                            in0=ps[:].rearrange(
                                "p (h w) -> p h w", w=64),
                            in1=bvr,
                            op=mybir.AluOpType.add)

            # ---- phase 3: attention, one unit = (pair, q-half) ----
            expp = tc.alloc_tile_pool(name="exps", bufs=8)
            normp = tc.alloc_tile_pool(name="norm", bufs=2)

            def attention_unit(p, half, j):
                hA, hB = 2 * p, 2 * p + 1
                cj = half * 1024 + j * 512
                psoA = ps_o.tile([128, 512], F32, tag="ps_o", name="psoA")
                psoB = ps_o.tile([128, 512], F32, tag="ps_o", name="psoB")

                def emit_attnv(kk, ea, eb):
                    nc.tensor.matmul(
                        psoA[:], lhsT=vt[kk][:, hA * 128:(hA + 1) * 128],
                        rhs=ea, start=(kk == 0), stop=(kk == NT - 1))
                    nc.tensor.matmul(
                        psoB[:], lhsT=vt[kk][:, hB * 128:(hB + 1) * 128],
                        rhs=eb, start=(kk == 0), stop=(kk == NT - 1))

                pend = []
                for k in range(NT):
                    pssA = ps_s.tile([128, 512], F32, tag="pss",
                                     name="pssA")
                    pssB = ps_s.tile([128, 512], F32, tag="pss",
                                     name="pssB")
                    # row-tiled pair: head A in PE rows 0:64, head B in
                    # rows 64:128 (explicit tile_position)
                    nc.tensor.matmul(
                        pssA[:], lhsT=kt2[p][0:64, k * 128:(k + 1) * 128],
                        rhs=qt2[p][0:64, cj:cj + 512],
                        start=True, stop=True, tile_position=(0, 0))
                    nc.tensor.matmul(
                        pssB[:], lhsT=kt2[p][64:128, k * 128:(k + 1) * 128],
                        rhs=qt2[p][64:128, cj:cj + 512],
                        start=True, stop=True, tile_position=(64, 0))
                    eaps = []
                    for hi, pss in ((0, pssA), (1, pssB)):
                        if (2 * k + hi) % 3 == 2:
                            esd = expp.tile([128, 512], I16, tag="esd",
                                            name="esd")
                            nc.vector.tensor_scalar(
                                out=esd[:], in0=pss[:],
                                scalar1=float(SCHRA_A * 0.125),
                                scalar2=mask16_sb[:, k:k + 1],
                                op0=mybir.AluOpType.mult,
                                op1=mybir.AluOpType.add)
                            eaps.append(esd[:].bitcast(DT))
                        else:
                            es = expp.tile([128, 512], DT, tag="es",
                                           name="es")
                            nc.scalar.activation(
                                out=es[:], in_=pss[:],
                                func=mybir.ActivationFunctionType.Exp,
                                bias=mask_sb[:, k:k + 1], scale=0.125)
                            eaps.append(es[:])
                    pend.append((k, eaps[0], eaps[1]))
                    # depth-2 software pipeline: attn@V for k-2 issues
                    # after scores(k), so the PE queue never blocks on
                    # exp(k) results
                    if len(pend) > 2:
                        emit_attnv(*pend.pop(0))
                for item in pend:
                    emit_attnv(*item)
                # normalize: row 0 = sum(exp), rows 64..128 = O^T
                for hi, pso in ((0, psoA), (1, psoB)):
                    ri = normp.tile([1, 512], F32, tag="ri", name="ri")
                    nc.vector.reciprocal_approx_fast(ri[:], pso[0:1, :])
                    oto = normp.tile([64, 512], F32, tag="oto", name="oto")
                    nc.scalar.copy(out=oto[:], in_=pso[64:128, :])
                    rep = normp.tile([64, 512], F32, tag="rep", name="rep")
                    nc.gpsimd.partition_broadcast(rep[:], ri[0:1, :])
                    nc.vector.tensor_mul(
                        xtl[p][hi * 64:(hi + 1) * 64, cj:cj + 512],
                        oto[:], rep[:])

            # ---- phase 4: output projection (partial) ----
            wop = tc.alloc_tile_pool(name="wo", bufs=1)
            osbp = tc.alloc_tile_pool(name="osb", bufs=2)
            wo_sb = [wop.tile([128, E], DT, tag=f"wo{e}", name=f"wo{e}")
                     for e in range(NFT)]
            for e in range(NFT):
                nc.scalar.dma_start(out=wo_sb[e][:],
                                    in_=woT[e * 128:(e + 1) * 128, :])

            def final_proj(js):
                for j in js:
                    ob = osbp.tile([128, E], DT, tag="ob", name="ob")
                    for c2 in range(2):
                        ps = ps_s.tile([128, 512], F32, tag="pss",
                                       name="psf")
                        for e in range(NFT):
                            nc.tensor.matmul(
                                ps[:],
                                lhsT=xtl[e][:, j * 128:(j + 1) * 128],
                                rhs=wo_sb[e][:, c2 * 512:(c2 + 1) * 512],
                                start=(e == 0), stop=(e == NFT - 1))
                        nc.vector.tensor_tensor(
                            out=ob[:, c2 * 512:(c2 + 1) * 512], in0=ps[:],
                            in1=bo_sb[:, c2 * 512:(c2 + 1) * 512],
                            op=mybir.AluOpType.add)
                    nc.sync.dma_start(out=out[j * 128:(j + 1) * 128, :],
                                      in_=ob[:])

            for p in range(NP):
                for j in range(2):
                    attention_unit(p, 0, j)
            final_proj(range(NT // 2))
            for p in range(NP):
                for j in range(2):
                    attention_unit(p, 1, j)
            final_proj(range(NT // 2, NT))
            for pl in (osbp, wop, normp, expp, xlp, wp):
                pl.release()

    nc.compile()
    return nc


_NC_CACHE = None


def _get_nc():
    global _NC_CACHE
    if _NC_CACHE is None:
        _NC_CACHE = build_nc()
    return _NC_CACHE


def make_in_maps(query, key_, value, mask, w_q, b_q, w_k, b_k, w_v, b_v,
                 w_o, b_o):
    import ml_dtypes
    f32 = np.float32
    bf16 = ml_dtypes.bfloat16
    c = lambda a: np.ascontiguousarray(a).astype(bf16)
    in_maps = []
    for core in range(N_CORES):
        b, g = core // 2, core % 2
        fs = slice(g * FL, (g + 1) * FL)
        mb = np.where(mask[b], 0.0, -30.0).astype(f32)
        bo_full = (b_o.astype(f32, copy=False) if g == 0
                   else np.zeros(E, f32))
        in_maps.append({
            "qT": c(query[b].T.astype(f32, copy=False)),
            "kT": c(key_[b].T.astype(f32, copy=False)),
            "vT": c(value[b].T.astype(f32, copy=False)),
            "wqT": c(w_q[fs, :].T.astype(f32, copy=False)),
            "wkT": c(w_k[fs, :].T.astype(f32, copy=False)),
            "wvT": c(w_v[fs, :].T.astype(f32, copy=False)),
            "woT": c(w_o[:, fs].T.astype(f32, copy=False)),
            "bq": np.ascontiguousarray(
                b_q[fs].astype(f32, copy=False).reshape(NFT, 128).T),
            "bk": np.ascontiguousarray(
                b_k[fs].astype(f32, copy=False).reshape(NFT, 128).T),
            "bv2d": np.broadcast_to(
                b_v[fs].reshape(1, FL), (128, FL)).astype(bf16),
            "bo2d": np.broadcast_to(
                bo_full.reshape(1, E), (128, E)).astype(bf16),
            "maskb": np.ascontiguousarray(mb.reshape(NT, 128).T),
            "maskb16": np.ascontiguousarray(
                (mb * SCHRA_A + SCHRA_B).astype(f32).reshape(NT, 128).T),
            "vones": np.ones((128, HL), bf16),
        })
    return in_maps


def kernel(query=None, key_=None, value=None, mask=None, w_q=None, b_q=None,
           w_k=None, b_k=None, w_v=None, b_v=None, w_o=None, b_o=None,
           key=None, **_kwargs):
    if key_ is None:
        key_ = key
    args = [np.asarray(a) for a in
            (query, key_, value, mask, w_q, b_q, w_k, b_k, w_v, b_v,
             w_o, b_o)]
    nc = _get_nc()
    in_maps = make_in_maps(*args)
    res = run_bass_kernel_spmd(nc, in_maps, core_ids=list(range(N_CORES)))
    outs = [np.asarray(res.results[i]["out"], dtype=np.float32)
            for i in range(N_CORES)]
    full = np.empty((B, T, E), np.float32)
    for b in range(B):
        full[b] = outs[2 * b] + outs[2 * b + 1]
    return full


# revision 13
# speedup vs baseline: 1.0258x; 1.0258x over previous
"""Multi-head attention Trainium2 Bass kernel — iteration 2: paired heads.

Shapes (hardcoded): B=4, T=2048, E=1024, H=16, DK=64.
Sharding over 8 cores: core c -> (batch b = c//2, head-group g = c%2).

Key change vs iteration 1: heads are stored in PAIRS — qt2[p]/kt2[p]
[128, T] hold head 2p in partitions 0:64 and head 2p+1 in 64:128. The
two heads' score matmuls (contraction = 64) run CONCURRENTLY in the PE
array via row tiling (tile_position rows 0/64), halving score cost.
attn@V keeps the per-head M=65 ones-row form (denominator in PSUM row
64). Exp splits between ACT (exact) and DVE (Schraudolph int16/bf16
bit trick), tunable fraction.
"""

import numpy as np

import concourse.bass as bass
import concourse.tile as tile
from concourse import bacc, mybir
from concourse.bass_utils import run_bass_kernel_spmd

F32 = mybir.dt.float32
I16 = mybir.dt.int16
BF16 = mybir.dt.bfloat16
DT = BF16

B, T, E, H = 4, 2048, 1024, 16
DK = E // H            # 64
N_CORES = 8
FL = 512               # local f (8 heads * 64)
HL = 8                 # heads per core
NP = 4                 # head pairs per core
NT = T // 128          # 16 t-tiles
NE = E // 128          # 8 e-tiles
NFT = FL // 128        # 4 local f-tiles (one per head pair)
NC4 = T // 512         # 4 t-chunks of 512

SCHRA_A = 128.0 / np.log(2.0)
SCHRA_B = 16256.0 - 0.0579 * 128.0

# which key-tiles use the DVE Schraudolph exp (rest use exact ACT exp)
DVE_EXP_KS = frozenset((2, 5, 8, 11, 14))


def build_nc():
    nc = bacc.Bacc("TRN2", target_bir_lowering=False, debug=False,
                   enable_asserts=False)

    qT = nc.dram_tensor("qT", [E, T], DT, kind="ExternalInput").ap()
    kT = nc.dram_tensor("kT", [E, T], DT, kind="ExternalInput").ap()
    vT = nc.dram_tensor("vT", [E, T], DT, kind="ExternalInput").ap()
    wqT = nc.dram_tensor("wqT", [E, FL], DT, kind="ExternalInput").ap()
    wkT = nc.dram_tensor("wkT", [E, FL], DT, kind="ExternalInput").ap()
    wvT = nc.dram_tensor("wvT", [E, FL], DT, kind="ExternalInput").ap()
    woT = nc.dram_tensor("woT", [FL, E], DT, kind="ExternalInput").ap()
    bq = nc.dram_tensor("bq", [128, NFT], F32, kind="ExternalInput").ap()
    bk = nc.dram_tensor("bk", [128, NFT], F32, kind="ExternalInput").ap()
    bv2d = nc.dram_tensor("bv2d", [128, FL], DT, kind="ExternalInput").ap()
    bo2d = nc.dram_tensor("bo2d", [128, E], DT, kind="ExternalInput").ap()
    maskb = nc.dram_tensor("maskb", [128, NT], F32, kind="ExternalInput").ap()
    maskb16 = nc.dram_tensor("maskb16", [128, NT], F32,
                             kind="ExternalInput").ap()
    vones = nc.dram_tensor("vones", [128, HL], DT, kind="ExternalInput").ap()
    out = nc.dram_tensor("out", [T, E], DT, kind="ExternalOutput").ap()

    with tile.TileContext(nc) as tc:
        with (
            tc.tile_pool(name="const", bufs=1) as constp,
            tc.tile_pool(name="qkt", bufs=1) as qktp,
            tc.tile_pool(name="vsb", bufs=1) as vsbp,
            tc.tile_pool(name="xtl", bufs=1) as xtlp,
            tc.tile_pool(name="ps_s", bufs=5, space="PSUM") as ps_s,
            tc.tile_pool(name="ps_o", bufs=3, space="PSUM") as ps_o,
        ):
            # ---- constants (scalar ring keeps sync ring free) ----
            bq_sb = constp.tile([128, NFT], F32, tag="bq")
            nc.scalar.dma_start(out=bq_sb[:], in_=bq)
            bk_sb = constp.tile([128, NFT], F32, tag="bk")
            nc.scalar.dma_start(out=bk_sb[:], in_=bk)
            bv_sb = constp.tile([128, FL], DT, tag="bv2d")
            nc.scalar.dma_start(out=bv_sb[:], in_=bv2d)
            bo_sb = constp.tile([128, E], DT, tag="bo2d")
            nc.scalar.dma_start(out=bo_sb[:], in_=bo2d)
            mask_sb = constp.tile([128, NT], F32, tag="maskb")
            nc.scalar.dma_start(out=mask_sb[:], in_=maskb)
            mask16_sb = constp.tile([128, NT], F32, tag="maskb16")
            nc.scalar.dma_start(out=mask16_sb[:], in_=maskb16)

            # paired persistent activations: qt2[p]/kt2[p] rows 0:64 =
            # head 2p, rows 64:128 = head 2p+1 (f-tile p of the local 512)
            qt2 = [qktp.tile([128, T], DT, tag=f"qt{i}", name=f"qt{i}")
                   for i in range(NP)]
            kt2 = [qktp.tile([128, T], DT, tag=f"kt{i}", name=f"kt{i}")
                   for i in range(NP)]
            # V per t-tile: [128, 8 heads * 128]; per head: col 0 = 1.0
            # (row-sum trick: the softmax denominator lands in PSUM
            # partition 0, where reciprocal_approx_fast and
            # partition_broadcast work), cols 64..128 = V, rest zeros
            # (engine partition windows must be 0- or 64-based)
            vt = [vsbp.tile([128, HL * 128], DT, tag=f"v{j}", name=f"v{j}")
                  for j in range(NT)]
            for j in range(NT):
                nc.scalar.memzero(vt[j][:])
                nc.scalar.dma_start(
                    out=vt[j].rearrange("p (h w) -> p h w", w=128)[:, :, 0:1],
                    in_=vones.rearrange("p (h o) -> p h o", o=1))
            xtl = [xtlp.tile([128, T], DT, tag=f"x{i}", name=f"x{i}")
                   for i in range(NFT)]

            # ---- phase 1: K/Q projections, chunk-major (x loaded once) ----
            wp = tc.alloc_tile_pool(name="wqk", bufs=1)
            xlp = tc.alloc_tile_pool(name="xload", bufs=3)

            w_sb = {}
            for name, wdram in (("k", wkT), ("q", wqT)):
                w_sb[name] = [
                    wp.tile([128, FL], DT, tag=f"w{name}{e}",
                            name=f"w{name}{e}") for e in range(NE)]

            for name in ("k", "q"):
                wdram = wkT if name == "k" else wqT
                for e in range(NE):
                    nc.sync.dma_start(
                        out=w_sb[name][e][:],
                        in_=wdram[e * 128:(e + 1) * 128, :])
                xdram = kT if name == "k" else qT
                bias_sb = bk_sb if name == "k" else bq_sb
                dst = kt2 if name == "k" else qt2
                for c in range(NC4):
                    xe = xlp.tile([128, NE * 512], DT, tag="xchunk",
                                  name="xchunk")
                    nc.sync.dma_start(
                        out=xe.rearrange("p (e t) -> p e t", e=NE),
                        in_=xdram.rearrange("(e p) t -> p e t", p=128)
                        [:, :, c * 512:(c + 1) * 512])
                    for f in range(4):
                        ps = ps_s.tile([128, 512], F32, tag="pss",
                                       name="psqk")
                        for e in range(NE):
                            nc.tensor.matmul(
                                ps[:],
                                lhsT=w_sb[name][e][:,
                                                   f * 128:(f + 1) * 128],
                                rhs=xe[:, e * 512:(e + 1) * 512],
                                start=(e == 0), stop=(e == NE - 1))
                        nc.scalar.add(
                            dst[f][:, c * 512:(c + 1) * 512],
                            ps[:], bias_sb[:, f:f + 1])

            # ---- phase 2: V projection (bias via DVE tensor_tensor) ----
            with tc.tile_pool(name="wv", bufs=1) as wvp, \
                 tc.tile_pool(name="vload", bufs=1) as vlp:
                wv_sb = [wvp.tile([128, FL], DT, tag=f"wv{e}", name=f"wv{e}")
                         for e in range(NE)]
                for e in range(NE):
                    nc.scalar.dma_start(out=wv_sb[e][:],
                                        in_=wvT[e * 128:(e + 1) * 128, :])
                bvr = bv_sb.rearrange("p (h w) -> p h w", w=64)
                for hf in range(2):
                    vf = [vlp.tile([128, 1024], DT, tag=f"vf{e}",
                                   name=f"vf{e}") for e in range(NE)]
                    for e in range(NE):
                        nc.sync.dma_start(
                            out=vf[e][:],
                            in_=vT[e * 128:(e + 1) * 128,
                                   hf * 1024:(hf + 1) * 1024])
                    for jj in range(NT // 2):
                        j = hf * (NT // 2) + jj
                        ps = ps_s.tile([128, FL], F32, tag="pss",
                                       name="psv")
                        for e in range(NE):
                            nc.tensor.matmul(
                                ps[:],
                                lhsT=vf[e][:, jj * 128:(jj + 1) * 128],
                                rhs=wv_sb[e][:],
                                start=(e == 0), stop=(e == NE - 1))
                        nc.vector.tensor_tensor(
                            out=vt[j].rearrange(
                                "p (h w) -> p h w", w=128)[:, :, 64:128],
                            in0=ps[:].rearrange(
                                "p (h w) -> p h w", w=64),
                            in1=bvr,
                            op=mybir.AluOpType.add)

            # ---- phase 3: attention, one unit = (pair, q-half) ----
            expp = tc.alloc_tile_pool(name="exps", bufs=8)
            normp = tc.alloc_tile_pool(name="norm", bufs=2)

            def attention_unit(p, half, j):
                hA, hB = 2 * p, 2 * p + 1
                cj = half * 1024 + j * 512
                psoA = ps_o.tile([128, 512], F32, tag="ps_o", name="psoA")
                psoB = ps_o.tile([128, 512], F32, tag="ps_o", name="psoB")

                def emit_attnv(kk, ea, eb):
                    nc.tensor.matmul(
                        psoA[:], lhsT=vt[kk][:, hA * 128:(hA + 1) * 128],
                        rhs=ea, start=(kk == 0), stop=(kk == NT - 1))
                    nc.tensor.matmul(
                        psoB[:], lhsT=vt[kk][:, hB * 128:(hB + 1) * 128],
                        rhs=eb, start=(kk == 0), stop=(kk == NT - 1))

                pend = []
                for k in range(NT):
                    pssA = ps_s.tile([128, 512], F32, tag="pss",
                                     name="pssA")
                    pssB = ps_s.tile([128, 512], F32, tag="pss",
                                     name="pssB")
                    # row-tiled pair: head A in PE rows 0:64, head B in
                    # rows 64:128 (explicit tile_position)
                    nc.tensor.matmul(
                        pssA[:], lhsT=kt2[p][0:64, k * 128:(k + 1) * 128],
                        rhs=qt2[p][0:64, cj:cj + 512],
                        start=True, stop=True, tile_position=(0, 0))
                    nc.tensor.matmul(
                        pssB[:], lhsT=kt2[p][64:128, k * 128:(k + 1) * 128],
                        rhs=qt2[p][64:128, cj:cj + 512],
                        start=True, stop=True, tile_position=(64, 0))
                    eaps = []
                    for hi, pss in ((0, pssA), (1, pssB)):
                        if (2 * k + hi) % 8 in (2, 5, 7):
                            esd = expp.tile([128, 512], I16, tag="esd",
                                            name="esd")
                            nc.vector.tensor_scalar(
                                out=esd[:], in0=pss[:],
                                scalar1=float(SCHRA_A * 0.125),
                                scalar2=mask16_sb[:, k:k + 1],
                                op0=mybir.AluOpType.mult,
                                op1=mybir.AluOpType.add)
                            eaps.append(esd[:].bitcast(DT))
                        else:
                            es = expp.tile([128, 512], DT, tag="es",
                                           name="es")
                            nc.scalar.activation(
                                out=es[:], in_=pss[:],
                                func=mybir.ActivationFunctionType.Exp,
                                bias=mask_sb[:, k:k + 1], scale=0.125)
                            eaps.append(es[:])
                    pend.append((k, eaps[0], eaps[1]))
                    # depth-2 software pipeline: attn@V for k-2 issues
                    # after scores(k), so the PE queue never blocks on
                    # exp(k) results
                    if len(pend) > 2:
                        emit_attnv(*pend.pop(0))
                for item in pend:
                    emit_attnv(*item)
                # normalize: row 0 = sum(exp), rows 64..128 = O^T
                for hi, pso in ((0, psoA), (1, psoB)):
                    ri = normp.tile([1, 512], F32, tag="ri", name="ri")
                    nc.vector.reciprocal_approx_fast(ri[:], pso[0:1, :])
                    oto = normp.tile([64, 512], F32, tag="oto", name="oto")
                    nc.scalar.copy(out=oto[:], in_=pso[64:128, :])
                    rep = normp.tile([64, 512], F32, tag="rep", name="rep")
                    nc.gpsimd.partition_broadcast(rep[:], ri[0:1, :])
                    nc.vector.tensor_mul(
                        xtl[p][hi * 64:(hi + 1) * 64, cj:cj + 512],
                        oto[:], rep[:])

            # ---- phase 4: output projection (partial) ----
            wop = tc.alloc_tile_pool(name="wo", bufs=1)
            osbp = tc.alloc_tile_pool(name="osb", bufs=2)
            wo_sb = [wop.tile([128, E], DT, tag=f"wo{e}", name=f"wo{e}")
                     for e in range(NFT)]
            for e in range(NFT):
                nc.scalar.dma_start(out=wo_sb[e][:],
                                    in_=woT[e * 128:(e + 1) * 128, :])

            def final_proj(js):
                for j in js:
                    ob = osbp.tile([128, E], DT, tag="ob", name="ob")
                    for c2 in range(2):
                        ps = ps_s.tile([128, 512], F32, tag="pss",
                                       name="psf")
                        for e in range(NFT):
                            nc.tensor.matmul(
                                ps[:],
                                lhsT=xtl[e][:, j * 128:(j + 1) * 128],
                                rhs=wo_sb[e][:, c2 * 512:(c2 + 1) * 512],
                                start=(e == 0), stop=(e == NFT - 1))
                        nc.vector.tensor_tensor(
                            out=ob[:, c2 * 512:(c2 + 1) * 512], in0=ps[:],
                            in1=bo_sb[:, c2 * 512:(c2 + 1) * 512],
                            op=mybir.AluOpType.add)
                    nc.sync.dma_start(out=out[j * 128:(j + 1) * 128, :],
                                      in_=ob[:])

            for p in range(NP):
                for j in range(2):
                    attention_unit(p, 0, j)
            final_proj(range(NT // 2))
            for p in range(NP):
                for j in range(2):
                    attention_unit(p, 1, j)
            final_proj(range(NT // 2, NT))
            for pl in (osbp, wop, normp, expp, xlp, wp):
                pl.release()

    nc.compile()
    return nc


_NC_CACHE = None


def _get_nc():
    global _NC_CACHE
    if _NC_CACHE is None:
        _NC_CACHE = build_nc()
    return _NC_CACHE


def make_in_maps(query, key_, value, mask, w_q, b_q, w_k, b_k, w_v, b_v,
                 w_o, b_o):
    import ml_dtypes
    f32 = np.float32
    bf16 = ml_dtypes.bfloat16
    c = lambda a: np.ascontiguousarray(a).astype(bf16)
    in_maps = []
    for core in range(N_CORES):
        b, g = core // 2, core % 2
        fs = slice(g * FL, (g + 1) * FL)
        mb = np.where(mask[b], 0.0, -30.0).astype(f32)
        bo_full = (b_o.astype(f32, copy=False) if g == 0
                   else np.zeros(E, f32))
        in_maps.append({
            "qT": c(query[b].T.astype(f32, copy=False)),
            "kT": c(key_[b].T.astype(f32, copy=False)),
            "vT": c(value[b].T.astype(f32, copy=False)),
            "wqT": c(w_q[fs, :].T.astype(f32, copy=False)),
            "wkT": c(w_k[fs, :].T.astype(f32, copy=False)),
            "wvT": c(w_v[fs, :].T.astype(f32, copy=False)),
            "woT": c(w_o[:, fs].T.astype(f32, copy=False)),
            "bq": np.ascontiguousarray(
                b_q[fs].astype(f32, copy=False).reshape(NFT, 128).T),
            "bk": np.ascontiguousarray(
                b_k[fs].astype(f32, copy=False).reshape(NFT, 128).T),
            "bv2d": np.broadcast_to(
                b_v[fs].reshape(1, FL), (128, FL)).astype(bf16),
            "bo2d": np.broadcast_to(
                bo_full.reshape(1, E), (128, E)).astype(bf16),
            "maskb": np.ascontiguousarray(mb.reshape(NT, 128).T),
            "maskb16": np.ascontiguousarray(
                (mb * SCHRA_A + SCHRA_B).astype(f32).reshape(NT, 128).T),
            "vones": np.ones((128, HL), bf16),
        })
    return in_maps


def kernel(query=None, key_=None, value=None, mask=None, w_q=None, b_q=None,
           w_k=None, b_k=None, w_v=None, b_v=None, w_o=None, b_o=None,
           key=None, **_kwargs):
    if key_ is None:
        key_ = key
    args = [np.asarray(a) for a in
            (query, key_, value, mask, w_q, b_q, w_k, b_k, w_v, b_v,
             w_o, b_o)]
    nc = _get_nc()
    in_maps = make_in_maps(*args)
    res = run_bass_kernel_spmd(nc, in_maps, core_ids=list(range(N_CORES)))
    outs = [np.asarray(res.results[i]["out"], dtype=np.float32)
            for i in range(N_CORES)]
    full = np.empty((B, T, E), np.float32)
    for b in range(B):
        full[b] = outs[2 * b] + outs[2 * b + 1]
    return full
